# revision 32
# baseline (speedup 1.0000x reference)
"""Distance-weighted embedding loss on 8 Trainium2 NeuronCores.

reference:
    gathered = embedding[indices]                      # [B, K, D]
    sq = sum((gathered - emb_batch[:,None,:])**2, -1)  # [B, K]
    loss = sum(sq * attr_sim) / B                      # scalar

Sharding: data-parallel over the batch. Each of the 8 cores handles
B/8 = 512 samples; the embedding table is replicated (bf16). The host
adds the 8 partial sums and divides by B.

v9: PE-trace formulation. With G = gathered rows (bf16), X = per-sample
emb_batch rows, W = attr weights:

    loss_c = tr(sum_j WG_j^T G_j) + tr(sum_j (-2X)^T WG_j)
             + sum_b ||x_b||^2 sum_k w_bk

The only full-size elementwise op is WG = w (.) G on the DVE. It runs
in 2x mode via a pair-duplicated weight table: the DVE 2x mode only
requires the innermost AP dim be unit-stride 2-byte pairs, so weights
are stored duplicated ([w w] per k) and broadcast over d via a
stride-0 MIDDLE dim: in1 ap = [[2, nc], [0, 64], [1, 2]].

The two trace terms accumulate in PSUM on the otherwise-idle PE:
  - T1: one matmul per chunk (lhsT = WG chunk, rhs = G chunk)
  - T2: one 512-wide matmul per 4 chunks (lhsT = -2X block, rhs = WG)
Epilogue extracts PSUM diagonals with a mask, adds the ||x||^2 term,
and collapses partitions with a ones-matmul.
"""

import ml_dtypes
import numpy as np

import concourse.bass as bass
import concourse.tile as tile
from concourse import bacc, mybir
from concourse.bass_utils import run_bass_kernel_spmd

F32 = mybir.dt.float32
BF16 = mybir.dt.bfloat16
I32 = mybir.dt.int32

NCORES = 8
D = 128
P = 128
NCOL = 40

# per-block segment lists (col offset, width)
SEGS_FIRST = [(0, 10), (10, 20), (30, 20)]
SEGS_MID = [(0, 26), (26, 24)]
SEGS_LAST = [(0, 40), (40, 10)]
N_WARM = 20  # dummy matmuls bridging startup so the PE pstate-ramps


def build_program(V: int, S_C: int, K: int):
    G = S_C // P
    assert S_C % P == 0

    nc = bacc.Bacc("TRN2", target_bir_lowering=False, debug=False)

    offs = nc.dram_tensor("offsets", [P, G * K], I32, kind="ExternalInput")
    table = nc.dram_tensor("embedding", [V, D], BF16, kind="ExternalInput")
    negx2 = nc.dram_tensor("negx2", [P, G * D], BF16, kind="ExternalInput")
    attr = nc.dram_tensor("attr_sim", [P, G * K], BF16, kind="ExternalInput")
    attr2 = nc.dram_tensor("attr2", [P, G * K * 2], BF16,
                           kind="ExternalInput")
    xg = nc.dram_tensor("xg", [P, G * D], BF16, kind="ExternalInput")
    dmask = nc.dram_tensor("dmask", [P, D], F32, kind="ExternalInput")
    loss = nc.dram_tensor("loss", [1, 1], F32, kind="ExternalOutput")

    def segs_of(g):
        if g == 0:
            return SEGS_FIRST
        if g == G - 1:
            return SEGS_LAST
        return SEGS_MID

    with tile.TileContext(nc) as tc:
        with (
            tc.tile_pool(name="const", bufs=1) as const,
            tc.tile_pool(name="gather", bufs=9) as gpool,
            tc.tile_pool(name="wg", bufs=9) as wpool,
            tc.tile_pool(name="psum", bufs=1, space="PSUM") as psum,
        ):
            # segment-0 offsets in their own tiny tile -> first gather
            # gated only by this small load on the sync queue
            w0 = SEGS_FIRST[0][1]
            offs0 = const.tile([P, w0], I32)
            nc.sync.dma_start(out=offs0[:], in_=offs[:, :w0])
            offs_sb = const.tile([P, G * K], I32)
            nc.sync.dma_start(out=offs_sb[:, w0:], in_=offs[:, w0:])

            attr2_sb = const.tile([P, G * K * 2], BF16)
            nc.scalar.dma_start(out=attr2_sb[:], in_=attr2[:])
            negx2_sb = const.tile([P, G * D], BF16)
            nc.scalar.dma_start(out=negx2_sb[:], in_=negx2[:])
            attr_sb = const.tile([P, G * K], BF16)
            nc.scalar.dma_start(out=attr_sb[:], in_=attr[:])
            xg_sb = const.tile([P, G * D], BF16)
            nc.scalar.dma_start(out=xg_sb[:], in_=xg[:])
            dmask_sb = const.tile([P, D], F32)
            nc.scalar.dma_start(out=dmask_sb[:], in_=dmask[:])

            # T3 = sum_g (sum_d x^2) * (sum_k w) per partition: depends only
            # on const loads, so compute it up front, off the critical tail
            xsq = const.tile([P, G * D], F32)
            nc.vector.tensor_tensor(out=xsq[:], in0=xg_sb[:], in1=xg_sb[:],
                                    op=mybir.AluOpType.mult)
            xsr = const.tile([P, G], F32)
            nc.vector.tensor_reduce(
                out=xsr[:], in_=xsq[:].rearrange("p (g d) -> p g d", g=G),
                axis=mybir.AxisListType.X, op=mybir.AluOpType.add)
            wsr = const.tile([P, G], F32)
            nc.vector.tensor_reduce(
                out=wsr[:], in_=attr_sb[:].rearrange("p (g k) -> p g k", g=G),
                axis=mybir.AxisListType.X, op=mybir.AluOpType.add)
            t3g = const.tile([P, G], F32)
            nc.vector.tensor_tensor(out=t3g[:], in0=xsr[:], in1=wsr[:],
                                    op=mybir.AluOpType.mult)
            t3r = const.tile([P, 1], F32)
            nc.vector.tensor_reduce(out=t3r[:], in_=t3g[:],
                                    axis=mybir.AxisListType.X,
                                    op=mybir.AluOpType.add)

            ones = const.tile([P, 1], F32)
            nc.vector.memset(ones[:], 1.0)

            ps1 = psum.tile([P, D], F32)
            ps2 = psum.tile([P, 4 * D], F32)

            # PE pstate warmup: the tensor engine only reaches full clock
            # after ~3us of continuous execution. Run dummy matmuls on a
            # memset tile through the startup window so the real matmul
            # stream arrives at a warmed PE.
            warm = const.tile([P, 4 * D], BF16)
            nc.vector.memset(warm[:], 0.25)
            psw = psum.tile([P, 4 * D], F32)
            for _ in range(N_WARM):
                nc.tensor.matmul(out=psw[:], lhsT=warm[:, :D], rhs=warm[:],
                                 start=True, stop=True,
                                 skip_group_check=True)

            all_segs = [(g, k0, w) for g in range(G) for k0, w in segs_of(g)]
            n = len(all_segs)
            nch_tot = sum(w for _, _, w in all_segs)
            n_t2 = sum((w + 3) // 4 for _, _, w in all_segs)
            LOOK = 7
            gathered = {}
            wgs = {}

            def do_gather(i):
                g, k0, nc_t = all_segs[i]
                mm_full = gpool.tile([P, NCOL * D], BF16, tag="m")
                mm = mm_full[:, :nc_t * D]
                if i == 0:
                    off_ap = offs0[:, :nc_t]
                else:
                    off_ap = offs_sb[:, g * K + k0: g * K + k0 + nc_t]
                nc.gpsimd.indirect_dma_start(
                    out=mm,
                    out_offset=None,
                    in_=table[:],
                    in_offset=bass.IndirectOffsetOnAxis(ap=off_ap, axis=0),
                )
                gathered[i] = mm

            def do_weight(i):
                g, k0, nc_t = all_segs[i]
                mm = gathered[i]
                wt_full = wpool.tile([P, NCOL * D], BF16, tag="w")
                wt = wt_full[:, :nc_t * D]
                c0 = (g * K + k0) * 2
                a2 = attr2_sb[:, c0:c0 + nc_t * 2]
                # [P, nc, 2] -> [P, nc, 1, 2] -> stride-0 broadcast on dim 2:
                # innermost stays unit-stride 2-byte pairs => DVE 2x mode
                a2b = a2.rearrange("p (n two) -> p n two", two=2).unsqueeze(
                    2).to_broadcast([P, nc_t, D // 2, 2])
                with nc.allow_low_precision("bf16 weighted rows feed fp32 "
                                            "psum accumulation"):
                    nc.vector.tensor_tensor(
                        out=wt.rearrange("p (n h two) -> p n h two", two=2,
                                         n=nc_t),
                        in0=mm.rearrange("p (n h two) -> p n h two", two=2,
                                         n=nc_t),
                        in1=a2b,
                        op=mybir.AluOpType.mult,
                    )
                wgs[i] = wt

            # all buffers resident: issue every gather up front (the Pool
            # queue pipelines desc-gen; no tile-recycle waits)
            for i in range(n):
                do_gather(i)
            for i in range(min(LOOK, n)):
                do_weight(i)

            ch_idx = 0
            t2_idx = 0
            for i, (g, k0, nc_t) in enumerate(all_segs):
                if i + LOOK < n:
                    do_weight(i + LOOK)

                mm = gathered.pop(i)
                wt = wgs.pop(i)
                xb = negx2_sb[:, g * D:(g + 1) * D]
                last_seg = i == n - 1

                def emit_t1():
                    nonlocal ch_idx
                    for t in range(nc_t):
                        nc.tensor.matmul(
                            out=ps1[:],
                            lhsT=wt[:, t * D:(t + 1) * D],
                            rhs=mm[:, t * D:(t + 1) * D],
                            start=ch_idx == 0, stop=ch_idx == nch_tot - 1,
                        )
                        ch_idx += 1

                def emit_t2():
                    nonlocal t2_idx
                    for q0 in range(0, nc_t, 4):
                        qw = min(4, nc_t - q0)
                        nc.tensor.matmul(
                            out=ps2[:, :qw * D],
                            lhsT=xb,
                            rhs=wt[:, q0 * D:(q0 + qw) * D],
                            start=t2_idx == 0, stop=t2_idx == n_t2 - 1,
                            skip_group_check=True,
                        )
                        t2_idx += 1

                # last segment: finish ps2 first so its (longer) readout
                # overlaps the final T1 matmuls
                if last_seg:
                    emit_t2()
                    d2 = const.tile([P, 4 * D], F32)
                    nc.vector.tensor_tensor(
                        out=d2[:].rearrange("p (q d) -> p q d", q=4),
                        in0=ps2[:].rearrange("p (q d) -> p q d", q=4),
                        in1=dmask_sb[:].unsqueeze(1).to_broadcast([P, 4, D]),
                        op=mybir.AluOpType.mult)
                    t2r = const.tile([P, 1], F32)
                    nc.vector.tensor_reduce(out=t2r[:], in_=d2[:],
                                            axis=mybir.AxisListType.X,
                                            op=mybir.AluOpType.add)
                    # t2r + t3r combine overlaps the remaining T1 matmuls
                    t23 = const.tile([P, 1], F32)
                    nc.vector.tensor_tensor(out=t23[:], in0=t2r[:],
                                            in1=t3r[:],
                                            op=mybir.AluOpType.add)
                    emit_t1()
                else:
                    emit_t1()
                    emit_t2()

            # ---- epilogue ----
            d1 = const.tile([P, D], F32)
            nc.vector.tensor_tensor(out=d1[:], in0=ps1[:], in1=dmask_sb[:],
                                    op=mybir.AluOpType.mult)
            t1r = const.tile([P, 1], F32)
            nc.vector.tensor_reduce(out=t1r[:], in_=d1[:],
                                    axis=mybir.AxisListType.X,
                                    op=mybir.AluOpType.add)

            tot = const.tile([P, 1], F32)
            nc.vector.tensor_tensor(out=tot[:], in0=t1r[:], in1=t23[:],
                                    op=mybir.AluOpType.add)

            psf = psum.tile([1, 1], F32)
            nc.tensor.matmul(out=psf[:], lhsT=ones[:], rhs=tot[:],
                             start=True, stop=True)
            out_sb = const.tile([1, 1], F32)
            nc.vector.tensor_copy(out=out_sb[:], in_=psf[:])
            nc.sync.dma_start(out=loss[:], in_=out_sb[:])

    nc.compile()
    return nc


def shard_inputs(emb_batch, embedding, attr_sim, indices, ncores=NCORES):
    """Build the per-core input maps (layout prep only)."""
    B, K = attr_sim.shape
    s_c = B // ncores
    g = s_c // P
    BF = ml_dtypes.bfloat16
    x_f = np.asarray(emb_batch, dtype=np.float32)
    attr_bf = np.asarray(attr_sim, dtype=np.float32).astype(BF)
    emb_bf = np.asarray(embedding, dtype=np.float32).astype(BF)
    idx = np.asarray(indices).astype(np.int32)

    dmask = np.eye(P, D, dtype=np.float32)

    in_maps = []
    for c in range(ncores):
        sl = slice(c * s_c, (c + 1) * s_c)
        idx_c = idx[sl]  # [s_c, K]
        offs = np.ascontiguousarray(
            idx_c.reshape(g, P, K).transpose(1, 0, 2).reshape(P, g * K)
        )
        x_c = x_f[sl].reshape(g, P, D).transpose(1, 0, 2)  # [P, G, D]
        negx2 = np.ascontiguousarray(
            (-2.0 * x_c).astype(BF).reshape(P, g * D))
        xg = np.ascontiguousarray(x_c.astype(BF).reshape(P, g * D))
        attr_c = np.ascontiguousarray(
            attr_bf[sl].reshape(g, P, K).transpose(1, 0, 2).reshape(P, g * K))
        attr2 = np.ascontiguousarray(np.repeat(attr_c, 2, axis=1))
        in_maps.append({
            "offsets": offs,
            "embedding": emb_bf,
            "negx2": negx2,
            "attr_sim": attr_c,
            "attr2": attr2,
            "xg": xg,
            "dmask": dmask,
        })
    return in_maps


_cached = {}


def kernel(emb_batch, embedding, attr_sim, indices, beta):
    emb_batch = np.asarray(emb_batch)
    embedding = np.asarray(embedding)
    attr_sim = np.asarray(attr_sim)
    indices = np.asarray(indices)
    B, K = attr_sim.shape
    V = embedding.shape[0]
    key = (V, B // NCORES, K)
    if key not in _cached:
        _cached[key] = build_program(V, B // NCORES, K)
    nc = _cached[key]
    in_maps = shard_inputs(emb_batch, embedding, attr_sim, indices)
    res = run_bass_kernel_spmd(nc, in_maps, list(range(NCORES)))
    partials = [np.asarray(res.results[c]["loss"], dtype=np.float64).sum()
                for c in range(NCORES)]
    return np.float32(np.sum(np.asarray(partials, dtype=np.float64)) / B)


# revision 33
# speedup vs baseline: 1.1343x; 1.1343x over previous
"""Distance-weighted embedding loss on 8 Trainium2 NeuronCores.

reference:
    gathered = embedding[indices]                      # [B, K, D]
    sq = sum((gathered - emb_batch[:,None,:])**2, -1)  # [B, K]
    loss = sum(sq * attr_sim) / B                      # scalar

Sharding: data-parallel over the batch. Each of the 8 cores handles
B/8 = 512 samples; the embedding table is replicated (bf16). The host
adds the 8 partial sums and divides by B.

v9: PE-trace formulation. With G = gathered rows (bf16), X = per-sample
emb_batch rows, W = attr weights:

    loss_c = tr(sum_j WG_j^T G_j) + tr(sum_j (-2X)^T WG_j)
             + sum_b ||x_b||^2 sum_k w_bk

The only full-size elementwise op is WG = w (.) G on the DVE. It runs
in 2x mode via a pair-duplicated weight table: the DVE 2x mode only
requires the innermost AP dim be unit-stride 2-byte pairs, so weights
are stored duplicated ([w w] per k) and broadcast over d via a
stride-0 MIDDLE dim: in1 ap = [[2, nc], [0, 64], [1, 2]].

The two trace terms accumulate in PSUM on the otherwise-idle PE:
  - T1: one matmul per chunk (lhsT = WG chunk, rhs = G chunk)
  - T2: one 512-wide matmul per 4 chunks (lhsT = -2X block, rhs = WG)
Epilogue extracts PSUM diagonals with a mask, adds the ||x||^2 term,
and collapses partitions with a ones-matmul.
"""

import ml_dtypes
import numpy as np

import concourse.bass as bass
import concourse.tile as tile
from concourse import bacc, mybir
from concourse.bass_utils import run_bass_kernel_spmd

F32 = mybir.dt.float32
BF16 = mybir.dt.bfloat16
I32 = mybir.dt.int32

NCORES = 8
D = 128
P = 128
NCOL = 40

# per-block segment lists (col offset, width)
SEGS_FIRST = [(0, 10), (10, 20), (30, 20)]
SEGS_MID = [(0, 26), (26, 24)]
SEGS_LAST = [(0, 40), (40, 10)]
N_WARM = 32  # dummy matmuls bridging startup so the PE pstate-ramps


def build_program(V: int, S_C: int, K: int):
    G = S_C // P
    assert S_C % P == 0

    nc = bacc.Bacc("TRN2", target_bir_lowering=False, debug=False)

    offs = nc.dram_tensor("offsets", [P, G * K], I32, kind="ExternalInput")
    table = nc.dram_tensor("embedding", [V, D], BF16, kind="ExternalInput")
    negx2 = nc.dram_tensor("negx2", [P, G * D], BF16, kind="ExternalInput")
    attr = nc.dram_tensor("attr_sim", [P, G * K], BF16, kind="ExternalInput")
    attr2 = nc.dram_tensor("attr2", [P, G * K * 2], BF16,
                           kind="ExternalInput")
    xg = nc.dram_tensor("xg", [P, G * D], BF16, kind="ExternalInput")
    dmask = nc.dram_tensor("dmask", [P, D], F32, kind="ExternalInput")
    loss = nc.dram_tensor("loss", [1, 1], F32, kind="ExternalOutput")

    def segs_of(g):
        if g == 0:
            return SEGS_FIRST
        if g == G - 1:
            return SEGS_LAST
        return SEGS_MID

    with tile.TileContext(nc) as tc:
        with (
            tc.tile_pool(name="const", bufs=1) as const,
            tc.tile_pool(name="gather", bufs=9) as gpool,
            tc.tile_pool(name="wg", bufs=9) as wpool,
            tc.tile_pool(name="psum", bufs=1, space="PSUM") as psum,
        ):
            # segment-0 offsets in their own tiny tile -> first gather
            # gated only by this small load on the sync queue
            w0 = SEGS_FIRST[0][1]
            offs0 = const.tile([P, w0], I32)
            nc.sync.dma_start(out=offs0[:], in_=offs[:, :w0])
            offs_sb = const.tile([P, G * K], I32)
            nc.sync.dma_start(out=offs_sb[:, w0:], in_=offs[:, w0:])

            attr2_sb = const.tile([P, G * K * 2], BF16)
            nc.scalar.dma_start(out=attr2_sb[:], in_=attr2[:])
            negx2_sb = const.tile([P, G * D], BF16)
            nc.scalar.dma_start(out=negx2_sb[:], in_=negx2[:])
            attr_sb = const.tile([P, G * K], BF16)
            nc.scalar.dma_start(out=attr_sb[:], in_=attr[:])
            xg_sb = const.tile([P, G * D], BF16)
            nc.scalar.dma_start(out=xg_sb[:], in_=xg[:])
            dmask_sb = const.tile([P, D], F32)
            nc.scalar.dma_start(out=dmask_sb[:], in_=dmask[:])

            # T3 = sum_g (sum_d x^2) * (sum_k w) per partition: depends only
            # on const loads, so compute it up front, off the critical tail
            xsq = const.tile([P, G * D], F32)
            nc.vector.tensor_tensor(out=xsq[:], in0=xg_sb[:], in1=xg_sb[:],
                                    op=mybir.AluOpType.mult)
            xsr = const.tile([P, G], F32)
            nc.vector.tensor_reduce(
                out=xsr[:], in_=xsq[:].rearrange("p (g d) -> p g d", g=G),
                axis=mybir.AxisListType.X, op=mybir.AluOpType.add)
            wsr = const.tile([P, G], F32)
            nc.vector.tensor_reduce(
                out=wsr[:], in_=attr_sb[:].rearrange("p (g k) -> p g k", g=G),
                axis=mybir.AxisListType.X, op=mybir.AluOpType.add)
            t3g = const.tile([P, G], F32)
            nc.vector.tensor_tensor(out=t3g[:], in0=xsr[:], in1=wsr[:],
                                    op=mybir.AluOpType.mult)
            t3r = const.tile([P, 1], F32)
            nc.vector.tensor_reduce(out=t3r[:], in_=t3g[:],
                                    axis=mybir.AxisListType.X,
                                    op=mybir.AluOpType.add)

            ones = const.tile([P, 1], F32)
            nc.vector.memset(ones[:], 1.0)

            ps1 = psum.tile([P, D], F32)
            ps2 = psum.tile([P, 4 * D], F32)

            # PE pstate warmup: the tensor engine only reaches full clock
            # after ~3us of continuous execution. Run dummy matmuls on a
            # memset tile through the startup window so the real matmul
            # stream arrives at a warmed PE.
            warm = const.tile([P, 4 * D], BF16)
            nc.vector.memset(warm[:], 0.25)
            psw = psum.tile([P, 4 * D], F32)
            for _ in range(N_WARM):
                nc.tensor.matmul(out=psw[:], lhsT=warm[:, :D], rhs=warm[:],
                                 start=True, stop=True,
                                 skip_group_check=True)

            all_segs = [(g, k0, w) for g in range(G) for k0, w in segs_of(g)]
            n = len(all_segs)
            nch_tot = sum(w for _, _, w in all_segs)
            n_t2 = sum((w + 3) // 4 for _, _, w in all_segs)
            LOOK = 5
            gathered = {}
            wgs = {}

            def do_gather(i):
                g, k0, nc_t = all_segs[i]
                mm_full = gpool.tile([P, NCOL * D], BF16, tag="m")
                mm = mm_full[:, :nc_t * D]
                if i == 0:
                    off_ap = offs0[:, :nc_t]
                else:
                    off_ap = offs_sb[:, g * K + k0: g * K + k0 + nc_t]
                nc.gpsimd.indirect_dma_start(
                    out=mm,
                    out_offset=None,
                    in_=table[:],
                    in_offset=bass.IndirectOffsetOnAxis(ap=off_ap, axis=0),
                )
                gathered[i] = mm

            def do_weight(i):
                g, k0, nc_t = all_segs[i]
                mm = gathered[i]
                wt_full = wpool.tile([P, NCOL * D], BF16, tag="w")
                wt = wt_full[:, :nc_t * D]
                c0 = (g * K + k0) * 2
                a2 = attr2_sb[:, c0:c0 + nc_t * 2]
                # [P, nc, 2] -> [P, nc, 1, 2] -> stride-0 broadcast on dim 2:
                # innermost stays unit-stride 2-byte pairs => DVE 2x mode
                a2b = a2.rearrange("p (n two) -> p n two", two=2).unsqueeze(
                    2).to_broadcast([P, nc_t, D // 2, 2])
                with nc.allow_low_precision("bf16 weighted rows feed fp32 "
                                            "psum accumulation"):
                    nc.vector.tensor_tensor(
                        out=wt.rearrange("p (n h two) -> p n h two", two=2,
                                         n=nc_t),
                        in0=mm.rearrange("p (n h two) -> p n h two", two=2,
                                         n=nc_t),
                        in1=a2b,
                        op=mybir.AluOpType.mult,
                    )
                wgs[i] = wt

            # all buffers resident: issue every gather up front (the Pool
            # queue pipelines desc-gen; no tile-recycle waits)
            for i in range(n):
                do_gather(i)
            for i in range(min(LOOK, n)):
                do_weight(i)

            ch_idx = 0
            t2_idx = 0
            for i, (g, k0, nc_t) in enumerate(all_segs):
                if i + LOOK < n:
                    do_weight(i + LOOK)

                mm = gathered.pop(i)
                wt = wgs.pop(i)
                xb = negx2_sb[:, g * D:(g + 1) * D]
                last_seg = i == n - 1

                def emit_t1():
                    nonlocal ch_idx
                    for t in range(nc_t):
                        nc.tensor.matmul(
                            out=ps1[:],
                            lhsT=wt[:, t * D:(t + 1) * D],
                            rhs=mm[:, t * D:(t + 1) * D],
                            start=ch_idx == 0, stop=ch_idx == nch_tot - 1,
                        )
                        ch_idx += 1

                def emit_t2():
                    nonlocal t2_idx
                    for q0 in range(0, nc_t, 4):
                        qw = min(4, nc_t - q0)
                        nc.tensor.matmul(
                            out=ps2[:, :qw * D],
                            lhsT=xb,
                            rhs=wt[:, q0 * D:(q0 + qw) * D],
                            start=t2_idx == 0, stop=t2_idx == n_t2 - 1,
                            skip_group_check=True,
                        )
                        t2_idx += 1

                # last segment: finish ps2 first so its (longer) readout
                # overlaps the final T1 matmuls
                if last_seg:
                    emit_t2()
                    d2 = const.tile([P, 4 * D], F32)
                    nc.vector.tensor_tensor(
                        out=d2[:].rearrange("p (q d) -> p q d", q=4),
                        in0=ps2[:].rearrange("p (q d) -> p q d", q=4),
                        in1=dmask_sb[:].unsqueeze(1).to_broadcast([P, 4, D]),
                        op=mybir.AluOpType.mult)
                    t2r = const.tile([P, 1], F32)
                    nc.vector.tensor_reduce(out=t2r[:], in_=d2[:],
                                            axis=mybir.AxisListType.X,
                                            op=mybir.AluOpType.add)
                    # t2r + t3r combine overlaps the remaining T1 matmuls
                    t23 = const.tile([P, 1], F32)
                    nc.vector.tensor_tensor(out=t23[:], in0=t2r[:],
                                            in1=t3r[:],
                                            op=mybir.AluOpType.add)
                    emit_t1()
                else:
                    emit_t1()
                    emit_t2()

            # ---- epilogue ----
            d1 = const.tile([P, D], F32)
            nc.vector.tensor_tensor(out=d1[:], in0=ps1[:], in1=dmask_sb[:],
                                    op=mybir.AluOpType.mult)
            t1r = const.tile([P, 1], F32)
            nc.vector.tensor_reduce(out=t1r[:], in_=d1[:],
                                    axis=mybir.AxisListType.X,
                                    op=mybir.AluOpType.add)

            tot = const.tile([P, 1], F32)
            nc.vector.tensor_tensor(out=tot[:], in0=t1r[:], in1=t23[:],
                                    op=mybir.AluOpType.add)

            psf = psum.tile([1, 1], F32)
            nc.tensor.matmul(out=psf[:], lhsT=ones[:], rhs=tot[:],
                             start=True, stop=True)
            out_sb = const.tile([1, 1], F32)
            nc.vector.tensor_copy(out=out_sb[:], in_=psf[:])
            nc.sync.dma_start(out=loss[:], in_=out_sb[:])

    nc.compile()
    return nc


def shard_inputs(emb_batch, embedding, attr_sim, indices, ncores=NCORES):
    """Build the per-core input maps (layout prep only)."""
    B, K = attr_sim.shape
    s_c = B // ncores
    g = s_c // P
    BF = ml_dtypes.bfloat16
    x_f = np.asarray(emb_batch, dtype=np.float32)
    attr_bf = np.asarray(attr_sim, dtype=np.float32).astype(BF)
    emb_bf = np.asarray(embedding, dtype=np.float32).astype(BF)
    idx = np.asarray(indices).astype(np.int32)

    dmask = np.eye(P, D, dtype=np.float32)

    in_maps = []
    for c in range(ncores):
        sl = slice(c * s_c, (c + 1) * s_c)
        idx_c = idx[sl]  # [s_c, K]
        offs = np.ascontiguousarray(
            idx_c.reshape(g, P, K).transpose(1, 0, 2).reshape(P, g * K)
        )
        x_c = x_f[sl].reshape(g, P, D).transpose(1, 0, 2)  # [P, G, D]
        negx2 = np.ascontiguousarray(
            (-2.0 * x_c).astype(BF).reshape(P, g * D))
        xg = np.ascontiguousarray(x_c.astype(BF).reshape(P, g * D))
        attr_c = np.ascontiguousarray(
            attr_bf[sl].reshape(g, P, K).transpose(1, 0, 2).reshape(P, g * K))
        attr2 = np.ascontiguousarray(np.repeat(attr_c, 2, axis=1))
        in_maps.append({
            "offsets": offs,
            "embedding": emb_bf,
            "negx2": negx2,
            "attr_sim": attr_c,
            "attr2": attr2,
            "xg": xg,
            "dmask": dmask,
        })
    return in_maps


_cached = {}


def kernel(emb_batch, embedding, attr_sim, indices, beta):
    emb_batch = np.asarray(emb_batch)
    embedding = np.asarray(embedding)
    attr_sim = np.asarray(attr_sim)
    indices = np.asarray(indices)
    B, K = attr_sim.shape
    V = embedding.shape[0]
    key = (V, B // NCORES, K)
    if key not in _cached:
        _cached[key] = build_program(V, B // NCORES, K)
    nc = _cached[key]
    in_maps = shard_inputs(emb_batch, embedding, attr_sim, indices)
    res = run_bass_kernel_spmd(nc, in_maps, list(range(NCORES)))
    partials = [np.asarray(res.results[c]["loss"], dtype=np.float64).sum()
                for c in range(NCORES)]
    return np.float32(np.sum(np.asarray(partials, dtype=np.float64)) / B)
